# revision 1
# baseline (speedup 1.0000x reference)
"""Trainium2 Bass kernel for nn_AttentionBlock (Set-Transformer MAB block).

Reference computation (per batch b):
    Qp = Q @ Wq.T + bq ; Kp = K @ Wk.T + bk ; Vp = K @ Wv.T + bv   (4 heads of 64)
    A  = softmax(Qp Kp^T / 8)  ;  ctx = A Vp
    O  = LN0(Qp + ctx) ;  O = O + relu(O @ Wo.T + bo) ;  out = LN1(O)

Sharding: data-parallel over (batch, query-half) -> 8 independent shards,
one per NeuronCore, no collectives.  Each core sees its 1024 queries, the
full 2048 keys of its batch, and all weights.  Host-side sharding also
re-lays-out the inputs (zero-FLOP transposes): Q/K/W are shipped
feature-major so the kernel needs no on-chip input transposes.

Layout / scheduling choices:
  * scores are computed transposed (keys on partitions, ST[k,q]); the
    softmax denominator comes free from a ones-column appended to V in the
    ctx matmul (row 64 of ctxT = colsum of exp scores).  No max-subtraction
    (scores ~N(0,1), exp can't overflow).
  * ACT exp (1 elem/lane/cycle) is the pacing resource.  The head phase
    reaches the first score matmul fast; remaining projection work is
    drip-fed into PE slack during the attention loop via a filler queue.
    Per-head merge overlaps the next head's exps.  The LN/MLP tail is
    split across DVE/ACT/GPSIMD.
  * matmuls use float32r (full-rate fp32 streaming, ~1.5e-4 rel precision).
"""

from contextlib import ExitStack

import numpy as np

import concourse.bass as bass
import concourse.tile as tile
from concourse import bacc, mybir
from concourse.bass_utils import run_bass_kernel_spmd
from concourse.masks import make_identity

FP = mybir.dt.float32
FR = mybir.dt.float32r
AF = mybir.ActivationFunctionType
OP = mybir.AluOpType

B = 4
SQ_FULL = 2048   # queries per batch
SK = 2048        # keys per batch
D = 256
H = 4
DH = D // H      # 64
NCORES = 8
QSPLIT = 2
SQ = SQ_FULL // QSPLIT    # queries per core
NQT = SQ // 128           # 8 query tiles
NKT = SK // 128           # 16 key tiles
NDT = D // 128            # 2 feature tiles
LN_EPS = 1e-5
SCALE = 0.125             # 1 / sqrt(DH)

MT = FR  # dtype of matmul-feeding tiles (float32r)


def _emit(nc):
    QTd = nc.declare_dram_parameter("QT", [D, SQ], MT, isOutput=False)
    KTd = nc.declare_dram_parameter("KT", [D, SK], MT, isOutput=False)
    WTd = {
        n: nc.declare_dram_parameter(n, [D, D], MT, isOutput=False)
        for n in ("WqT", "WkT", "WvT", "WoT")
    }
    V1 = {
        n: nc.declare_dram_parameter(n, [D], FP, isOutput=False)
        for n in ("bq", "bk", "bv", "bo", "g0", "beta0", "g1", "beta1")
    }
    out = nc.declare_dram_parameter("out", [SQ, D], FP, isOutput=True)

    with tile.TileContext(nc) as tc, ExitStack() as ctx:
        singles = ctx.enter_context(tc.tile_pool(name="singles", bufs=1))
        big = ctx.enter_context(tc.tile_pool(name="big", bufs=1))
        ex = ctx.enter_context(tc.tile_pool(name="ex", bufs=3))
        ctp = ctx.enter_context(tc.tile_pool(name="ctp", bufs=2))
        tmp = ctx.enter_context(tc.tile_pool(name="tmp", bufs=6))
        outp = ctx.enter_context(tc.tile_pool(name="outp", bufs=4))

        ident = singles.tile([128, 128], FP)
        nc.vector.memset(ident[:], 0.0)
        make_identity(nc, ident, nomemset=True)
        epst = singles.tile([128, 1], FP)
        nc.vector.memset(epst, LN_EPS)
        ones41 = singles.tile([128, 4, 1], FP)
        nc.vector.memset(ones41[:], 1.0)
        onesF = singles.tile([1, 128], FP)
        nc.vector.memset(onesF[:], 1.0)

        def bcast(name):  # [D] dram -> [128, D] sbuf, partition-stride-0 DMA
            a = V1[name][:]
            t = singles.tile([128, D], FP, tag=f"bc_{name}")
            src = bass.AP(tensor=a.tensor, offset=a.offset, ap=[[0, 128]] + list(a.ap))
            nc.gpsimd.dma_start(out=t[:], in_=src)
            return t

        def ppart(name):  # [D] dram -> [128, NDT] sbuf (feature-on-partition)
            t = singles.tile([128, NDT], FP, tag=f"pp_{name}")
            nc.sync.dma_start(out=t[:], in_=V1[name][:].rearrange("(t p) -> p t", p=128))
            return t

        def layernorm(dst, src, g_b, b_b, gp_engine):
            st = tmp.tile([128, 6], FP, tag="st")
            mv = tmp.tile([128, 2], FP, tag="mv")
            nc.vector.bn_stats(out=st[:], in_=src)
            nc.vector.bn_aggr(out=mv[:], in_=st[:])
            sd = tmp.tile([128, 1], FP, tag="sd")
            nc.scalar.activation(out=sd[:], in_=mv[:, 1:2], func=AF.Sqrt, bias=epst[:])
            rs = tmp.tile([128, 1], FP, tag="rs")
            nc.vector.reciprocal(out=rs[:], in_=sd[:])
            nc.vector.tensor_scalar(
                out=dst, in0=src, scalar1=mv[:, 0:1], scalar2=rs[:],
                op0=OP.subtract, op1=OP.mult)
            gp_engine.tensor_mul(out=dst, in0=dst, in1=g_b[:])
            gp_engine.tensor_add(out=dst, in0=dst, in1=b_b[:])

        QpT = big.tile([128, NDT, SQ], MT)
        KpT = big.tile([128, NDT, SK], MT)
        Vp = big.tile([128, NKT, H, DH + 1], MT)
        O = big.tile([128, NQT, D], FP)
        recips = big.tile([128, NQT, H], FP)
        KT = big.tile([128, NDT, SK], MT)
        QT = big.tile([128, NDT, SQ], MT)
        WT = {}
        for wname in ("WqT", "WkT", "WvT", "WoT"):
            wt_tile = big.tile([128, NDT, D], MT, tag=f"wt_{wname}")
            WT[wname] = wt_tile

        # ========== phase A: loads + critical-path projections ==============
        with ExitStack() as pctx:
            mm_ps = pctx.enter_context(tc.tile_pool(name="mmps", bufs=4, space="PSUM"))

            # input DMAs spread across issue engines, ordered by first use:
            # gpsimd: Wq/Wk/Wv, bv, K chunks, Wo, remaining broadcasts;
            # sync: Q chunks + per-partition biases; ACT stays free for the
            # projection bias-moves that gate the first exp
            for wname in ("WqT", "WkT", "WvT"):
                nc.gpsimd.dma_start(
                    out=WT[wname][:],
                    in_=WTd[wname][:, :].rearrange("(s p) d -> p s d", p=128))
            for c in range(2):
                nc.sync.dma_start(
                    out=QT[:, :, c * 512:(c + 1) * 512],
                    in_=QTd[:, c * 512:(c + 1) * 512].rearrange("(s p) q -> p s q", p=128))
            bq_p = ppart("bq")
            bk_p = ppart("bk")
            bv_b = bcast("bv")
            bv_v = bv_b[:, :].rearrange("p (h d) -> p h d", h=H)
            for c in range(4):
                eng = nc.gpsimd if c % 2 == 0 else nc.sync
                eng.dma_start(
                    out=KT[:, :, c * 512:(c + 1) * 512],
                    in_=KTd[:, c * 512:(c + 1) * 512].rearrange("(s p) k -> p s k", p=128))
            nc.gpsimd.dma_start(
                out=WT["WoT"][:],
                in_=WTd["WoT"][:, :].rearrange("(s p) d -> p s d", p=128))
            aq = V1["bq"][:]
            bq_b = singles.tile([128, D], FP, tag="bc_bq")
            nc.sync.dma_start(
                out=bq_b[:],
                in_=bass.AP(tensor=aq.tensor, offset=aq.offset, ap=[[0, 128]] + list(aq.ap)))
            bo_b = bcast("bo")
            g0_b = bcast("g0")
            b0_b = bcast("beta0")
            g1_b = bcast("g1")
            b1_b = bcast("beta1")

            def proj_chunk(pool, dst, wt, src, bias_p, dvt, n, on_act):
                ps = pool.tile([128, 512], FP, tag=("mm" if pool is mm_ps else "fil"))
                for dqt in range(NDT):
                    nc.tensor.matmul(
                        ps[:],
                        wt[:, dqt, dvt * 128:(dvt + 1) * 128],
                        src[:, dqt, n * 512:(n + 1) * 512],
                        start=(dqt == 0), stop=(dqt == NDT - 1))
                if on_act:
                    nc.scalar.activation(
                        out=dst[:, dvt, n * 512:(n + 1) * 512], in_=ps[:],
                        func=AF.Identity, bias=bias_p[:, dvt:dvt + 1], scale=1.0)
                else:
                    nc.vector.tensor_scalar_add(
                        out=dst[:, dvt, n * 512:(n + 1) * 512], in0=ps[:],
                        scalar1=bias_p[:, dvt:dvt + 1])

            def vp_pair(kts, pool):  # V projection for a pair of key tiles
                for kt in kts:
                    ps = pool.tile([128, 512], FP, tag=("mm" if pool is mm_ps else "fil"))
                    for dqt in range(NDT):
                        nc.tensor.matmul(
                            ps[:, :D],
                            KT[:, dqt, kt * 128:(kt + 1) * 128],
                            WT["WvT"][:, dqt, :],
                            start=(dqt == 0), stop=(dqt == NDT - 1))
                    nc.vector.tensor_copy(out=Vp[:, kt, :, DH:DH + 1], in_=ones41[:])
                    nc.vector.tensor_add(
                        out=Vp[:, kt, :, 0:DH],
                        in0=ps[:, :D].rearrange("p (h d) -> p h d", h=H),
                        in1=bv_v)

            def obase(qt, pool):  # residual base O = Qp token-major
                ps = pool.tile([128, 512], FP, tag=("mm" if pool is mm_ps else "fil"))
                for dqt in range(NDT):
                    nc.tensor.matmul(
                        ps[:, :D],
                        QT[:, dqt, qt * 128:(qt + 1) * 128],
                        WT["WqT"][:, dqt, :],
                        start=(dqt == 0), stop=(dqt == NDT - 1))
                nc.vector.tensor_add(out=O[:, qt, :], in0=ps[:, :D], in1=bq_b[:])

            # critical path: QpT(dvt0), KpT(dvt0, keys 0..511), Vp(0..3)
            proj_chunk(mm_ps, QpT, WT["WqT"], QT, bq_p, 0, 0, True)
            proj_chunk(mm_ps, QpT, WT["WqT"], QT, bq_p, 0, 1, True)
            proj_chunk(mm_ps, KpT, WT["WkT"], KT, bk_p, 0, 0, True)
            vp_pair((0, 1), mm_ps)
            vp_pair((2, 3), mm_ps)

        # ========== phase B: attention + fillers ============================
        with ExitStack() as pctx:
            sc_ps = pctx.enter_context(tc.tile_pool(name="scps", bufs=2, space="PSUM"))
            cx_ps = pctx.enter_context(tc.tile_pool(name="cxps", bufs=1, space="PSUM"))
            aux_ps = pctx.enter_context(tc.tile_pool(name="auxps", bufs=2, space="PSUM"))

            # remaining projections, drip-fed into PE slack in dependency order
            fillers = []
            for c in range(1, 4):
                fillers.append(lambda c=c: proj_chunk(
                    aux_ps, KpT, WT["WkT"], KT, bk_p, 0, c, False))
                fillers.append(lambda c=c: vp_pair((c * 4, c * 4 + 1), aux_ps))
                fillers.append(lambda c=c: vp_pair((c * 4 + 2, c * 4 + 3), aux_ps))
            for n in range(SK // 512):
                fillers.append(lambda n=n: proj_chunk(
                    aux_ps, KpT, WT["WkT"], KT, bk_p, 1, n, False))
            for n in range(SQ // 512):
                fillers.append(lambda n=n: proj_chunk(
                    aux_ps, QpT, WT["WqT"], QT, bq_p, 1, n, False))
            for qt in range(NQT):
                fillers.append(lambda qt=qt: obase(qt, aux_ps))

            def pump(n):
                for _ in range(n):
                    if fillers:
                        fillers.pop(0)()

            for h in range(H):
                po = (h % 2) * DH
                dvt = h // 2

                def mm_s(kt):
                    sps = sc_ps.tile([128, SQ], FP, tag="sc")
                    for n in range(SQ // 512):
                        nc.tensor.matmul(
                            sps[:, n * 512:(n + 1) * 512],
                            KpT[po:po + DH, dvt, kt * 128:(kt + 1) * 128],
                            QpT[po:po + DH, dvt, n * 512:(n + 1) * 512],
                            start=True, stop=True)
                    return sps

                cps = cx_ps.tile([DH + 1, SQ], FP, tag="cx")
                sps = mm_s(0)
                for kt in range(NKT):
                    nxt = mm_s(kt + 1) if kt + 1 < NKT else None
                    e = ex.tile([128, SQ], MT, tag="ex")
                    nc.scalar.activation(out=e[:], in_=sps[:], func=AF.Exp, scale=SCALE)
                    for n in range(SQ // 512):
                        nc.tensor.matmul(
                            cps[:, n * 512:(n + 1) * 512],
                            Vp[:, kt, h, :],
                            e[:, n * 512:(n + 1) * 512],
                            start=(kt == 0), stop=(kt == NKT - 1))
                    pump(2 if h == 0 else 1)
                    sps = nxt

                # merge this head into O while the next head's exps run
                ctxTh = ctp.tile([DH + 1, SQ], FP, tag="ct")
                if h == H - 1:
                    nc.scalar.copy(out=ctxTh[:], in_=cps[:])
                else:
                    nc.vector.tensor_copy(out=ctxTh[:], in_=cps[:])
                for qt in range(NQT):
                    pmt = aux_ps.tile([128, DH + 1], FP, tag="fil")
                    nc.tensor.transpose(
                        pmt[:], ctxTh[:, qt * 128:(qt + 1) * 128],
                        ident[:DH + 1, :DH + 1])
                    nc.vector.reciprocal(
                        out=recips[:, qt, h:h + 1], in_=pmt[:, DH:DH + 1])
                    # O = ctx/colsum + Qp  (fused multiply-add)
                    nc.vector.scalar_tensor_tensor(
                        out=O[:, qt, h * DH:(h + 1) * DH],
                        in0=pmt[:, 0:DH],
                        scalar=recips[:, qt, h:h + 1],
                        in1=O[:, qt, h * DH:(h + 1) * DH],
                        op0=OP.mult, op1=OP.add)
                    if h == H - 1:
                        layernorm(O[:, qt, :], O[:, qt, :], g0_b, b0_b, nc.gpsimd)

        # ========== phase C: LN0, MLP, LN1, store ===========================
        with ExitStack() as pctx:
            mm_ps = pctx.enter_context(tc.tile_pool(name="mmps2", bufs=4, space="PSUM"))

            ones_row = singles.tile([1, 128], MT)
            nc.vector.tensor_copy(out=ones_row[:], in_=onesF[:])
            bo_row = singles.tile([1, D], MT)
            nc.vector.tensor_copy(out=bo_row[:], in_=bo_b[0:1, :])

            OT = big.tile([128, NDT, SQ], MT)
            for qt in range(NQT):
                ps = mm_ps.tile([128, 512], FP, tag="mm")
                for dvt in range(NDT):
                    nc.tensor.transpose(
                        ps[:, dvt * 128:(dvt + 1) * 128],
                        O[:, qt, dvt * 128:(dvt + 1) * 128], ident[:])
                nc.scalar.copy(
                    out=OT[:, :, qt * 128:(qt + 1) * 128],
                    in_=ps[:, :D].rearrange("p (t x) -> p t x", t=NDT))
            for qt in range(NQT):
                p4 = mm_ps.tile([128, 512], FP, tag="mm")
                for dvt in range(NDT):
                    nc.tensor.matmul(
                        p4[:, :D],
                        OT[:, dvt, qt * 128:(qt + 1) * 128],
                        WT["WoT"][:, dvt, :],
                        start=(dvt == 0), stop=False)
                nc.tensor.matmul(
                    p4[:, :D], ones_row[:], bo_row[:], start=False, stop=True)
                t1 = tmp.tile([128, D], FP, tag="t1")
                nc.scalar.activation(out=t1[:], in_=p4[:, :D], func=AF.Relu)
                nc.vector.tensor_add(out=O[:, qt, :], in0=O[:, qt, :], in1=t1[:])
                f = outp.tile([128, D], FP, tag="f")
                layernorm(f[:], O[:, qt, :], g1_b, b1_b, nc.gpsimd)
                deng = (nc.sync, nc.gpsimd, nc.scalar)[qt % 3]
                deng.dma_start(out=out[qt * 128:(qt + 1) * 128, :], in_=f[:])

    return nc


_NC = None


def build_nc():
    global _NC
    if _NC is None:
        nc = bacc.Bacc("TRN2", target_bir_lowering=False)
        _emit(nc)
        nc.compile()
        _NC = nc
    return _NC


def shard_inputs(Q, K, Wq, bq, Wk, bk, Wv, bv, Wo, bo, g0, beta0, g1, beta1):
    # host-side zero-FLOP layout transforms: ship everything feature-major
    shared = {
        "WqT": np.asarray(Wq, dtype=np.float32).T,
        "WkT": np.asarray(Wk, dtype=np.float32).T,
        "WvT": np.asarray(Wv, dtype=np.float32).T,
        "WoT": np.asarray(Wo, dtype=np.float32).T,
        "bq": bq, "bk": bk, "bv": bv, "bo": bo,
        "g0": g0, "beta0": beta0, "g1": g1, "beta1": beta1,
    }
    shared = {k: np.ascontiguousarray(v, dtype=np.float32) for k, v in shared.items()}
    in_maps = []
    for c in range(NCORES):
        b, half = c // QSPLIT, c % QSPLIT
        m = dict(shared)
        m["QT"] = np.ascontiguousarray(
            np.asarray(Q[b, half * SQ:(half + 1) * SQ, :], dtype=np.float32).T)
        m["KT"] = np.ascontiguousarray(np.asarray(K[b], dtype=np.float32).T)
        in_maps.append(m)
    return in_maps


def kernel(**inputs):
    nc = build_nc()
    in_maps = shard_inputs(**inputs)
    res = run_bass_kernel_spmd(nc, in_maps, core_ids=list(range(NCORES)))
    out = np.empty((B, SQ_FULL, D), np.float32)
    for c in range(NCORES):
        b, half = c // QSPLIT, c % QSPLIT
        out[b, half * SQ:(half + 1) * SQ, :] = res.results[c]["out"]
    return out



# revision 39
# speedup vs baseline: 6155.5373x; 6155.5373x over previous
"""Trainium2 Bass kernel for nn_AttentionBlock (Set-Transformer MAB block).

Reference computation (per batch b):
    Qp = Q @ Wq.T + bq ; Kp = K @ Wk.T + bk ; Vp = K @ Wv.T + bv   (4 heads of 64)
    A  = softmax(Qp Kp^T / 8)  ;  ctx = A Vp
    O  = LN0(Qp + ctx) ;  O = O + relu(O @ Wo.T + bo) ;  out = LN1(O)

Sharding: data-parallel over (batch, query-half) -> 8 independent shards,
one per NeuronCore, no collectives.  Each core sees its 1024 queries, the
full 2048 keys of its batch, and all weights, shipped feature-major and in
bf16 (host-side zero-FLOP transposes + dtype cast).

Layout / scheduling choices:
  * all matmul streams are bf16: fp32 moving operands stream at ~2 cycles/col
    on the PE while bf16 streams 1 col/cycle @2.4GHz, so bf16 halves PE time
    (accumulation stays fp32 in PSUM; measured end-to-end rel err ~5e-3,
    well inside the 2e-2 gate).
  * scores are computed transposed (keys on partitions, ST[k,q]); the
    softmax denominator comes free from a ones-column appended to V in the
    ctx matmul.  No max-subtraction (scores ~N(0,1), exp can't overflow).
  * ACT exp (1 elem/lane/cycle, dtype-independent) is the pacing resource:
    64 x [128,1024] exps ~ 86us.  Everything else is kept off ACT during
    the loop; projections drip-feed into PE slack via a filler queue.
  * bulk input DMAs go through the two HWDGE queues (sync/scalar, ~4KB
    packets); gpsimd's SWDGE path only carries the tiny bias broadcasts.
  * the LN0/MLP/LN1 tail is one fine-grained per-query-tile pipeline
    (merge -> stats -> normalize -> transpose -> Wo matmul -> relu-add ->
    LN1 -> store) spread over ACT/Vector, avoiding GpSimd elementwise
    (~4us per [128,256] pass on HW).
  * gamma/beta multiplies are skipped when the inputs are exactly
    ones/zeros (they are for this problem's setup_inputs); a general
    fallback variant applies them on Vector.
"""

from contextlib import ExitStack

import ml_dtypes
import numpy as np

import concourse.bass as bass
import concourse.tile as tile
from concourse import bacc, mybir
from concourse.bass_utils import run_bass_kernel_spmd
from concourse.masks import make_identity

FP = mybir.dt.float32
BF = mybir.dt.bfloat16
AF = mybir.ActivationFunctionType
OP = mybir.AluOpType

B = 4
SQ_FULL = 2048   # queries per batch
SK = 2048        # keys per batch
D = 256
H = 4
DH = D // H      # 64
NCORES = 8
QSPLIT = 2
SQ = SQ_FULL // QSPLIT    # queries per core
NQT = SQ // 128           # 8 query tiles
NKT = SK // 128           # 16 key tiles
NDT = D // 128            # 2 feature tiles
LN_EPS = 1e-5
SCALE = 0.125             # 1 / sqrt(DH)


def _emit(nc, skip_gb):
    QTd = nc.declare_dram_parameter("QT", [D, SQ], BF, isOutput=False)
    KTd = nc.declare_dram_parameter("KT", [D, SK], BF, isOutput=False)
    # weights are shipped partition-major ([p, s*d]) so each partition's data
    # is one contiguous 1KB run and packets coalesce across partitions
    WTd = {
        n: nc.declare_dram_parameter(n, [128, NDT * D], BF, isOutput=False)
        for n in ("WqT", "WkT", "WvT", "WoT")
    }
    V1 = {
        n: nc.declare_dram_parameter(n, [D], FP, isOutput=False)
        for n in ("bq", "bk", "bv", "bo", "g0", "beta0", "g1", "beta1")
    }
    out = nc.declare_dram_parameter("out", [SQ, D], FP, isOutput=True)

    with tile.TileContext(nc) as tc, ExitStack() as ctx:
        singles = ctx.enter_context(tc.tile_pool(name="singles", bufs=1))
        big = ctx.enter_context(tc.tile_pool(name="big", bufs=1))
        ex = ctx.enter_context(tc.tile_pool(name="ex", bufs=3))
        ctp = ctx.enter_context(tc.tile_pool(name="ctp", bufs=2))
        tmp = ctx.enter_context(tc.tile_pool(name="tmp", bufs=8))
        outp = ctx.enter_context(tc.tile_pool(name="outp", bufs=8))

        QpT = big.tile([128, NDT, SQ], BF)
        KpT = big.tile([128, NDT, SK], BF)
        Vp = big.tile([128, NKT, H, DH + 1], BF)
        O = big.tile([128, NQT, D], FP)
        OT = big.tile([128, NDT, SQ], BF)
        recips = big.tile([128, NQT, H], FP)
        KT = big.tile([128, NDT, SK], BF)
        QT = big.tile([128, NDT, SQ], BF)
        WT = {}
        for wname in ("WqT", "WkT", "WvT", "WoT"):
            wt_tile = big.tile([128, NDT, D], BF, tag=f"wt_{wname}")
            WT[wname] = wt_tile
        # tail stats (persist across the phase-C stage loops)
        mv0 = big.tile([128, NQT, 2], FP, tag="mv0")
        mv1 = big.tile([128, NQT, 2], FP, tag="mv1")
        sd8 = big.tile([128, 2, NQT], FP, tag="sd8")    # [ln, qt]
        rs8 = big.tile([128, 2, NQT], FP, tag="rs8")
        s18 = big.tile([128, 2, NQT], FP, tag="s18")

        ident = singles.tile([128, 128], FP)
        identB = singles.tile([128, 128], BF)
        epst = singles.tile([128, 1], FP)
        ones41 = singles.tile([128, 4, 1], FP)
        onesF = singles.tile([1, 128], FP)

        def bcast(name, eng):  # [D] dram -> [128, D] sbuf, partition-stride-0 DMA
            a = V1[name][:]
            t = singles.tile([128, D], FP, tag=f"bc_{name}")
            src = bass.AP(tensor=a.tensor, offset=a.offset, ap=[[0, 128]] + list(a.ap))
            eng.dma_start(out=t[:], in_=src)
            return t

        def ppart(name):  # [D] dram -> [128, NDT] sbuf (feature-on-partition)
            t = singles.tile([128, NDT], FP, tag=f"pp_{name}")
            nc.gpsimd.dma_start(out=t[:], in_=V1[name][:].rearrange("(t p) -> p t", p=128))
            return t

        # ========== phase A: loads + critical-path projections ==============
        with ExitStack() as pctx:
            mm_ps = pctx.enter_context(tc.tile_pool(name="mmps", bufs=4, space="PSUM"))

            # Bulk input DMAs go only through the two HWDGE queues
            # (sync/scalar): the gpsimd SWDGE path moves sub-1KB packets and
            # is several times slower.  GpSimd issues only the tiny
            # bias/broadcast loads.  Critical-first order per queue.
            def wload(name, eng):
                eng.dma_start(
                    out=WT[name][:],
                    in_=WTd[name][:, :].rearrange("p (s d) -> p s d", s=NDT))

            wload("WqT", nc.scalar)
            wload("WkT", nc.sync)
            nc.scalar.dma_start(
                out=QT[:, :, 0:512],
                in_=QTd[:, 0:512].rearrange("(s p) q -> p s q", p=128))
            nc.sync.dma_start(
                out=KT[:, :, 0:512],
                in_=KTd[:, 0:512].rearrange("(s p) k -> p s k", p=128))
            nc.scalar.dma_start(
                out=QT[:, :, 512:1024],
                in_=QTd[:, 512:1024].rearrange("(s p) q -> p s q", p=128))
            bq_p = ppart("bq")
            bk_p = ppart("bk")
            wload("WvT", nc.sync)
            bv_b = bcast("bv", nc.gpsimd)
            bv_v = bv_b[:, :].rearrange("p (h d) -> p h d", h=H)
            for c, eng in ((1, nc.scalar), (2, nc.sync), (3, nc.scalar)):
                eng.dma_start(
                    out=KT[:, :, c * 512:(c + 1) * 512],
                    in_=KTd[:, c * 512:(c + 1) * 512].rearrange("(s p) k -> p s k", p=128))
            wload("WoT", nc.sync)
            aq = V1["bq"][:]
            bq_b = singles.tile([128, D], FP, tag="bc_bq")
            nc.gpsimd.dma_start(
                out=bq_b[:],
                in_=bass.AP(tensor=aq.tensor, offset=aq.offset, ap=[[0, 128]] + list(aq.ap)))
            bo_b = bcast("bo", nc.gpsimd)
            if not skip_gb:
                g0_b = bcast("g0", nc.gpsimd)
                b0_b = bcast("beta0", nc.gpsimd)
                g1_b = bcast("g1", nc.gpsimd)
                b1_b = bcast("beta1", nc.gpsimd)

            # constants (emitted after the DMA issues so they don't delay them)
            nc.vector.memset(ident[:], 0.0)
            make_identity(nc, ident, nomemset=True)
            nc.vector.memset(identB[:], 0.0)
            make_identity(nc, identB, nomemset=True)
            nc.vector.memset(epst, LN_EPS)
            nc.vector.memset(ones41[:], 1.0)
            nc.vector.memset(onesF[:], 1.0)

            def proj_chunk(pool, dst, wt, src, bias_p, dvt, n, on_act):
                ps = pool.tile([128, 512], FP, tag=("mm" if pool is mm_ps else "fil"))
                for dqt in range(NDT):
                    nc.tensor.matmul(
                        ps[:],
                        wt[:, dqt, dvt * 128:(dvt + 1) * 128],
                        src[:, dqt, n * 512:(n + 1) * 512],
                        start=(dqt == 0), stop=(dqt == NDT - 1))
                if on_act:
                    nc.scalar.activation(
                        out=dst[:, dvt, n * 512:(n + 1) * 512], in_=ps[:],
                        func=AF.Identity, bias=bias_p[:, dvt:dvt + 1], scale=1.0)
                else:
                    nc.vector.tensor_scalar_add(
                        out=dst[:, dvt, n * 512:(n + 1) * 512], in0=ps[:],
                        scalar1=bias_p[:, dvt:dvt + 1])

            def vp_pair(kts, pool):  # V projection for a pair of key tiles
                for kt in kts:
                    ps = pool.tile([128, 512], FP, tag=("mm" if pool is mm_ps else "fil"))
                    for dqt in range(NDT):
                        nc.tensor.matmul(
                            ps[:, :D],
                            KT[:, dqt, kt * 128:(kt + 1) * 128],
                            WT["WvT"][:, dqt, :],
                            start=(dqt == 0), stop=(dqt == NDT - 1))
                    nc.vector.tensor_copy(out=Vp[:, kt, :, DH:DH + 1], in_=ones41[:])
                    nc.vector.tensor_add(
                        out=Vp[:, kt, :, 0:DH],
                        in0=ps[:, :D].rearrange("p (h d) -> p h d", h=H),
                        in1=bv_v)

            def obase(qt, pool):  # residual base O = Qp token-major
                ps = pool.tile([128, 512], FP, tag=("mm" if pool is mm_ps else "fil"))
                for dqt in range(NDT):
                    nc.tensor.matmul(
                        ps[:, :D],
                        QT[:, dqt, qt * 128:(qt + 1) * 128],
                        WT["WqT"][:, dqt, :],
                        start=(dqt == 0), stop=(dqt == NDT - 1))
                nc.vector.tensor_add(out=O[:, qt, :], in0=ps[:, :D], in1=bq_b[:])

            # critical path: QpT(dvt0), KpT(dvt0, keys 0..511), Vp(0..3).
            # (Keeping Vp here also keeps the PE busy through phase A, which
            # matters for the HAM clock-gate warmup.)
            proj_chunk(mm_ps, QpT, WT["WqT"], QT, bq_p, 0, 0, True)
            proj_chunk(mm_ps, QpT, WT["WqT"], QT, bq_p, 0, 1, True)
            proj_chunk(mm_ps, KpT, WT["WkT"], KT, bk_p, 0, 0, True)
            vp_pair((0, 1), mm_ps)
            vp_pair((2, 3), mm_ps)

        # ========== phase B: attention + fillers ============================
        with ExitStack() as pctx:
            sc_ps = pctx.enter_context(tc.tile_pool(name="scps", bufs=2, space="PSUM"))
            cx_ps = pctx.enter_context(tc.tile_pool(name="cxps", bufs=1, space="PSUM"))
            aux_ps = pctx.enter_context(tc.tile_pool(name="auxps", bufs=2, space="PSUM"))

            # remaining projections, drip-fed into PE slack in dependency
            # order.  The obase fillers MUST all be emitted before head 0's
            # merge loop (the merge reads+writes O), so they sit within the
            # first 17 pump slots (16 kts + one extra at kt 0).
            fillers = []
            for c in range(1, 4):
                fillers.append(lambda c=c: proj_chunk(
                    aux_ps, KpT, WT["WkT"], KT, bk_p, 0, c, False))
                fillers.append(lambda c=c: vp_pair((c * 4, c * 4 + 1), aux_ps))
                fillers.append(lambda c=c: vp_pair((c * 4 + 2, c * 4 + 3), aux_ps))
            for qt in range(NQT):
                fillers.append(lambda qt=qt: obase(qt, aux_ps))
            for n in range(SK // 512):
                fillers.append(lambda n=n: proj_chunk(
                    aux_ps, KpT, WT["WkT"], KT, bk_p, 1, n, False))
            for n in range(SQ // 512):
                fillers.append(lambda n=n: proj_chunk(
                    aux_ps, QpT, WT["WqT"], QT, bq_p, 1, n, False))

            def pump(n):
                for _ in range(n):
                    if fillers:
                        fillers.pop(0)()

            for h in range(H):
                po = (h % 2) * DH
                dvt = h // 2

                def mm_s(kt):
                    sps = sc_ps.tile([128, SQ], FP, tag="sc")
                    for n in range(SQ // 512):
                        nc.tensor.matmul(
                            sps[:, n * 512:(n + 1) * 512],
                            KpT[po:po + DH, dvt, kt * 128:(kt + 1) * 128],
                            QpT[po:po + DH, dvt, n * 512:(n + 1) * 512],
                            start=True, stop=True)
                    return sps

                cps = cx_ps.tile([DH + 1, SQ], FP, tag="cx")
                sps = mm_s(0)
                for kt in range(NKT):
                    nxt = mm_s(kt + 1) if kt + 1 < NKT else None
                    e = ex.tile([128, SQ], BF, tag="ex")
                    nc.scalar.activation(out=e[:], in_=sps[:], func=AF.Exp, scale=SCALE)
                    for n in range(SQ // 512):
                        nc.tensor.matmul(
                            cps[:, n * 512:(n + 1) * 512],
                            Vp[:, kt, h, :],
                            e[:, n * 512:(n + 1) * 512],
                            start=(kt == 0), stop=(kt == NKT - 1))
                    pump(2 if (h == 0 and kt == 0) else 1)
                    sps = nxt

                # merge this head into O while the next head's exps run.
                # (GpSimd cannot read PSUM, so copies and merges live on
                # Vector; the last head's copy is split with ACT, which is
                # done with exps by then.)
                ctxTh = ctp.tile([DH + 1, SQ], FP, tag="ct")
                if h == H - 1:
                    nc.vector.tensor_copy(out=ctxTh[:, 0:512], in_=cps[:, 0:512])
                    nc.scalar.copy(out=ctxTh[:, 512:SQ], in_=cps[:, 512:SQ])
                else:
                    nc.vector.tensor_copy(out=ctxTh[:], in_=cps[:])
                for qt in range(NQT):
                    pmt = aux_ps.tile([128, DH + 1], FP, tag="fil")
                    nc.tensor.transpose(
                        pmt[:], ctxTh[:, qt * 128:(qt + 1) * 128],
                        ident[:DH + 1, :DH + 1])
                    nc.vector.reciprocal_approx_fast(
                        out=recips[:, qt, h:h + 1], in_=pmt[:, DH:DH + 1])
                    # O = ctx/colsum + Qp  (fused multiply-add)
                    nc.vector.scalar_tensor_tensor(
                        out=O[:, qt, h * DH:(h + 1) * DH],
                        in0=pmt[:, 0:DH],
                        scalar=recips[:, qt, h:h + 1],
                        in1=O[:, qt, h * DH:(h + 1) * DH],
                        op0=OP.mult, op1=OP.add)
                    if h == H - 1:
                        # LN0 stats ride right behind the final merge so the
                        # tail chain starts as early as possible
                        st0 = tmp.tile([128, 6], FP, tag="st")
                        nc.vector.bn_stats(st0[:], O[:, qt, :])
                        nc.vector.bn_aggr(mv0[:, qt, :], st0[:])

        # ========== phase C: LN0, MLP, LN1, store (per-half pipelines) ======
        # GpSimd elementwise is ~4us per [128,256] pass on HW, so the tail
        # avoids it entirely: normalizes go to ACT (idle after the exps) with
        # a few on Vector.  The LN0 output is written as bf16 (ON) so the
        # feature-major transposes and the Wo matmul stream at bf16 rates.
        with ExitStack() as pctx:
            c_ps = pctx.enter_context(tc.tile_pool(name="cps2", bufs=2, space="PSUM"))

            ones_row = singles.tile([1, 128], BF)
            nc.vector.tensor_copy(out=ones_row[:], in_=onesF[:])
            bo_row = singles.tile([1, D], BF)
            nc.vector.tensor_copy(out=bo_row[:], in_=bo_b[0:1, :])

            def ln_stats(half, mv, ln):
                qb = half * 4
                for j in range(4):
                    st = tmp.tile([128, 6], FP, tag="st")
                    nc.vector.bn_stats(st[:], O[:, qb + j, :])
                    nc.vector.bn_aggr(mv[:, qb + j, :], st[:])
                nc.scalar.activation(
                    out=sd8[:, ln, qb:qb + 4], in_=mv[:, qb:qb + 4, 1],
                    func=AF.Sqrt, bias=epst[:])
                nc.vector.reciprocal_approx_fast(
                    out=rs8[:, ln, qb:qb + 4], in_=sd8[:, ln, qb:qb + 4])
                nc.vector.scalar_tensor_tensor(
                    out=s18[:, ln, qb:qb + 4], in0=mv[:, qb:qb + 4, 0],
                    scalar=-1.0, in1=rs8[:, ln, qb:qb + 4],
                    op0=OP.mult, op1=OP.mult)

            def normalize(dst, src, mv, ln, qt, eng):
                if eng is nc.scalar:
                    nc.scalar.activation(
                        out=dst, in_=src, func=AF.Identity,
                        scale=rs8[:, ln, qt:qt + 1], bias=s18[:, ln, qt:qt + 1])
                else:
                    eng.tensor_scalar(
                        out=dst, in0=src,
                        scalar1=mv[:, qt, 0:1], scalar2=rs8[:, ln, qt:qt + 1],
                        op0=OP.subtract, op1=OP.mult)

            # LN0 scale factors (stats were computed in the h3 merge loop)
            for half in range(2):
                qb = half * 4
                nc.scalar.activation(
                    out=sd8[:, 0, qb:qb + 4], in_=mv0[:, qb:qb + 4, 1],
                    func=AF.Sqrt, bias=epst[:])
                nc.vector.reciprocal_approx_fast(
                    out=rs8[:, 0, qb:qb + 4], in_=sd8[:, 0, qb:qb + 4])
                nc.vector.scalar_tensor_tensor(
                    out=s18[:, 0, qb:qb + 4], in0=mv0[:, qb:qb + 4, 0],
                    scalar=-1.0, in1=rs8[:, 0, qb:qb + 4],
                    op0=OP.mult, op1=OP.mult)

            # one fine-grained pipeline over query tiles: normalize -> 
            # transpose -> OT copy -> Wo matmul -> relu-add -> LN1 -> store,
            # engines interleaving across qt
            trs = [None, None]
            wos = [None, None]
            for qt in range(NQT):
                i, half = qt % 4, qt // 4
                qb = half * 4
                if i == 0:
                    tr_t = c_ps.tile([128, 1024], FP, tag="tr")
                    trs[half] = tr_t
                    wo_t = c_ps.tile([128, 1024], FP, tag="wo")
                    wos[half] = wo_t
                tr, wo = trs[half], wos[half]
                eng = (nc.scalar, nc.scalar, nc.vector, nc.scalar)[i]
                normalize(O[:, qt, :], O[:, qt, :], mv0, 0, qt, eng)
                if not skip_gb:
                    nc.vector.tensor_mul(out=O[:, qt, :], in0=O[:, qt, :], in1=g0_b[:])
                    nc.vector.tensor_add(out=O[:, qt, :], in0=O[:, qt, :], in1=b0_b[:])
                for dvt in range(NDT):
                    nc.tensor.transpose(
                        tr[:, i * 256 + dvt * 128:i * 256 + (dvt + 1) * 128],
                        O[:, qt, dvt * 128:(dvt + 1) * 128], ident[:])
                trv = tr[:, :].rearrange("p (i t q) -> p i t q", i=4, t=NDT)
                nc.scalar.copy(
                    out=OT[:, :, qt * 128:(qt + 1) * 128], in_=trv[:, i, :, :])
                for dvt in range(NDT):
                    nc.tensor.matmul(
                        wo[:, i * 256:(i + 1) * 256],
                        OT[:, dvt, qt * 128:(qt + 1) * 128],
                        WT["WoT"][:, dvt, :],
                        start=(dvt == 0), stop=False)
                nc.tensor.matmul(
                    wo[:, i * 256:(i + 1) * 256], ones_row[:], bo_row[:],
                    start=False, stop=True)
                if i % 2 == 1:
                    j = (i - 1) // 2
                    nc.vector.scalar_tensor_tensor(
                        out=O[:, qt - 1:qt + 1, :],
                        in0=wo[:, 512 * j:512 * (j + 1)], scalar=0.0,
                        in1=O[:, qt - 1:qt + 1, :], op0=OP.max, op1=OP.add)
                    for q2 in (qt - 1, qt):
                        st = tmp.tile([128, 6], FP, tag="st")
                        nc.vector.bn_stats(st[:], O[:, q2, :])
                        nc.vector.bn_aggr(mv1[:, q2, :], st[:])
                if i == 3:
                    nc.scalar.activation(
                        out=sd8[:, 1, qb:qb + 4], in_=mv1[:, qb:qb + 4, 1],
                        func=AF.Sqrt, bias=epst[:])
                    nc.vector.reciprocal_approx_fast(
                        out=rs8[:, 1, qb:qb + 4], in_=sd8[:, 1, qb:qb + 4])
                    nc.vector.scalar_tensor_tensor(
                        out=s18[:, 1, qb:qb + 4], in0=mv1[:, qb:qb + 4, 0],
                        scalar=-1.0, in1=rs8[:, 1, qb:qb + 4],
                        op0=OP.mult, op1=OP.mult)
                    for k in range(4):
                        q2 = qb + k
                        f = outp.tile([128, D], FP, tag="f")
                        e2 = (nc.scalar, nc.scalar, nc.vector, nc.scalar)[k]
                        normalize(f[:], O[:, q2, :], mv1, 1, q2, e2)
                        if not skip_gb:
                            nc.vector.tensor_mul(out=f[:], in0=f[:], in1=g1_b[:])
                            nc.vector.tensor_add(out=f[:], in0=f[:], in1=b1_b[:])
                        deng = (nc.sync, nc.gpsimd, nc.scalar, nc.sync)[k]
                        deng.dma_start(out=out[q2 * 128:(q2 + 1) * 128, :], in_=f[:])

    return nc


_NC = {}


def build_nc(skip_gb=True):
    if skip_gb not in _NC:
        nc = bacc.Bacc("TRN2", target_bir_lowering=False)
        _emit(nc, skip_gb)
        nc.compile()
        _NC[skip_gb] = nc
    return _NC[skip_gb]


def shard_inputs(Q, K, Wq, bq, Wk, bk, Wv, bv, Wo, bo, g0, beta0, g1, beta1):
    # host-side zero-FLOP layout transforms: ship everything feature-major bf16
    bf = ml_dtypes.bfloat16

    def wshape(w):  # [D, D] -> partition-major [128, NDT*D] (contiguous rows)
        wt = np.asarray(w).T.astype(bf)           # [ (s p), d ]
        return np.ascontiguousarray(
            wt.reshape(NDT, 128, D).transpose(1, 0, 2).reshape(128, NDT * D))

    shared = {
        "WqT": wshape(Wq),
        "WkT": wshape(Wk),
        "WvT": wshape(Wv),
        "WoT": wshape(Wo),
    }
    for n, v in (("bq", bq), ("bk", bk), ("bv", bv), ("bo", bo),
                 ("g0", g0), ("beta0", beta0), ("g1", g1), ("beta1", beta1)):
        shared[n] = np.ascontiguousarray(np.asarray(v, dtype=np.float32))
    in_maps = []
    for c in range(NCORES):
        b, half = c // QSPLIT, c % QSPLIT
        m = dict(shared)
        m["QT"] = np.ascontiguousarray(
            np.asarray(Q[b, half * SQ:(half + 1) * SQ, :]).T.astype(bf))
        m["KT"] = np.ascontiguousarray(np.asarray(K[b]).T.astype(bf))
        in_maps.append(m)
    return in_maps


def _gb_trivial(g0, beta0, g1, beta1):
    return bool(
        np.all(np.asarray(g0) == 1) and np.all(np.asarray(beta0) == 0)
        and np.all(np.asarray(g1) == 1) and np.all(np.asarray(beta1) == 0))


def kernel(**inputs):
    skip_gb = _gb_trivial(inputs["g0"], inputs["beta0"], inputs["g1"], inputs["beta1"])
    nc = build_nc(skip_gb)
    in_maps = shard_inputs(**inputs)
    res = run_bass_kernel_spmd(nc, in_maps, core_ids=list(range(NCORES)))
    out = np.empty((B, SQ_FULL, D), np.float32)
    for c in range(NCORES):
        b, half = c // QSPLIT, c % QSPLIT
        out[b, half * SQ:(half + 1) * SQ, :] = res.results[c]["out"]
    return out
